# revision 1
# baseline (speedup 1.0000x reference)
"""Trainium2 Bass kernel: Bahdanau local-p attention (B=32, S=2048, H=1024).

Sharding: data-parallel over batch. Each of the 8 cores processes B/8 = 4
batches end-to-end (weights replicated); no collectives.

Per-core dataflow (all matmul-heavy work in fp16 with fp32 PSUM accumulation):
  1. inputs[b] is cast fp32->fp16 into DRAM (SWDGE cast DMA), then loaded
     transposed into SBUF via the xbar DMA-transpose path -> inT [h, s].
  2. WH^T tiles [h'=128, s=512] = W_a-tile^T @ inT  (PE, fp16).
  3. tanh(WH^T + U_a h_t) fused on ACT (per-partition bias), fp16 out.
  4. score = v_a^T tanh(...) via M=1 matmuls accumulated over h'-tiles.
  5. softmax (minus-max) + gaussian window + 1/sum on DVE/ACT rows.
  6. weights row replicated to 128 partitions via ones-matmul; context^T
     computed on DVE with fused multiply+reduce against inT.
  7. final tanh([ctx, h_t] @ W_att) via fp16 matmuls, fp32 out.
"""

import math
from contextlib import ExitStack

import numpy as np

B, S, H, SIZE = 32, 2048, 1024, 1024
N_CORES = 8
BPC = B // N_CORES
P = 128
NB = 512

_compiled = None


def _build(bpc=BPC, s=S, h=H, size=SIZE, debug=False):
    import concourse.bacc as bacc
    import concourse.mybir as mybir
    import concourse.tile as tile

    F32 = mybir.dt.float32
    F16 = mybir.dt.float16
    AF = mybir.ActivationFunctionType
    ALU = mybir.AluOpType
    AX = mybir.AxisListType

    KT = h // P          # k-tiles over H
    SQ = s // NB         # s blocks of 512
    KT2 = 2 * h // P     # k-tiles over 2H (final projection)
    NO = size // NB      # output blocks
    H2 = h // 2
    denom = 2.0 * ((s // 2) / 2.0) ** 2
    inv_sq_denom = 1.0 / math.sqrt(denom)

    nc = bacc.Bacc("TRN2", target_bir_lowering=False, debug=debug)

    x = nc.dram_tensor("inputs", [bpc, s, h], F32, kind="ExternalInput").ap()
    W_p = nc.dram_tensor("W_p", [h, h], F32, kind="ExternalInput").ap()
    v_p = nc.dram_tensor("v_p", [h, 1], F32, kind="ExternalInput").ap()
    W_a = nc.dram_tensor("W_a", [h, h], F32, kind="ExternalInput").ap()
    U_a = nc.dram_tensor("U_a", [h, h], F32, kind="ExternalInput").ap()
    v_a = nc.dram_tensor("v_a", [h, 1], F32, kind="ExternalInput").ap()
    W_att = nc.dram_tensor("W_att", [2 * h, size], F32, kind="ExternalInput").ap()
    out = nc.dram_tensor("out", [bpc, size], F32, kind="ExternalOutput").ap()

    with tile.TileContext(nc) as tc, ExitStack() as ctx:
        dp = ctx.enter_context(tc.tile_pool(name="dram", bufs=2, space="DRAM"))
        sb = ctx.enter_context(tc.tile_pool(name="sb", bufs=1))
        ps = ctx.enter_context(tc.tile_pool(name="ps", bufs=1, space="PSUM"))

        inT_tiles = [None] * bpc

        def emit_input_dma(b):
            # single fully-contiguous fp32->fp16 cast DMA (SWDGE), then xbar
            # transposes of the k-th 128-column slice.
            xf = dp.tile([s, h], F16, name=f"xf16_{b}", tag="xf16")
            nc.gpsimd.dma_start(xf[:], x[b])
            inT = sb.tile([P, KT, s], F16, name=f"inT_{b}", tag="big", bufs=2)
            for k in range(KT):
                nc.sync.dma_start(inT[:, k, :], xf[:, k * P:(k + 1) * P],
                                  transpose=True)
            inT_tiles[b] = inT

        # ---- SWDGE queue order: b0 cast, U_a, W_a, v_a, b1 cast, W_p,
        # then b2/b3/W_att from the batch loop. HWDGE carries ONLY the xbar
        # transposes (mixing copy-DMAs with transposes serializes globally),
        # plus a few tiny DMAs emitted before the first transpose / at exit.
        def load_weight_f16(name, dram_ap, kt, n, tag, bufs):
            w16 = sb.tile([P, kt, n], F16, name=name, tag=tag, bufs=bufs)
            nc.gpsimd.dma_start(w16[:],
                                dram_ap.rearrange("(k p) n -> p k n", p=P))
            return w16

        htb = sb.tile([bpc, h], F32, name="htb", tag="htb")
        nc.scalar.dma_start(htb[:], x[:, s - 1, :])
        vp_rep = sb.tile([bpc, h], F32, name="vp_rep", tag="vp_rep")
        for i in range(bpc):
            nc.scalar.dma_start(vp_rep[i:i + 1, :], v_p.rearrange("n o -> o n"))
        emit_input_dma(0)
        ua_sb = load_weight_f16("ua_sb", U_a, KT, h, "ua", 1)
        wa_sb = load_weight_f16("wa_sb", W_a, KT, h, "wa", 1)
        va_sb = sb.tile([P, KT], F16, name="va_sb", tag="va")
        nc.gpsimd.dma_start(va_sb[:], v_a.rearrange("(k p) o -> p (k o)", p=P))
        emit_input_dma(1)
        wp_sb = load_weight_f16("wp_sb", W_p, KT, h, "wend", 1)

        # ---- constants ----
        ident_io = sb.tile([bpc, bpc], F32, name="ident_io", tag="ident_io")
        nc.gpsimd.iota(ident_io[:], pattern=[[1, bpc]], base=0,
                       channel_multiplier=-1,
                       allow_small_or_imprecise_dtypes=True)
        ident = sb.tile([bpc, bpc], F32, name="ident", tag="ident")
        nc.vector.tensor_scalar(ident[:], ident_io[:], 0.0, None,
                                op0=ALU.is_equal)
        ones1 = sb.tile([1, P], F16, name="ones1", tag="ones1")
        nc.vector.memset(ones1[:], 1.0)
        pos_row = sb.tile([1, s], F16, name="pos_row", tag="pos")
        nc.gpsimd.iota(pos_row[:], pattern=[[1, s]], base=0,
                       channel_multiplier=0,
                       allow_small_or_imprecise_dtypes=True)

        # ---- h_t extraction + transposes ----


        htT = sb.tile([P, KT, bpc], F32, name="htT", tag="htT")
        htT16 = sb.tile([P, KT, bpc], F16, name="htT16", tag="htT16")
        combT = sb.tile([P, KT2, bpc], F16, name="combT", tag="combT")
        for k in range(KT):
            pt = ps.tile([P, bpc], F32, name=f"pt_{k}", tag="wh", bufs=4)
            nc.tensor.transpose(pt[:], htb[:, k * P:(k + 1) * P], ident[:])
            nc.scalar.activation(htT[:, k, :], pt[:], AF.Copy)
            nc.vector.tensor_copy(htT16[:, k, :], pt[:])
            nc.vector.tensor_copy(combT[:, KT + k, :], pt[:])

        # ---- WT = h_t @ U_a, then transpose -> wtT ----
        wt_row = sb.tile([bpc, h], F32, name="wt_row", tag="wt_row")
        for n2 in range(h // NB):
            pwt = ps.tile([bpc, NB], F32, name=f"pwt_{n2}", tag="sc", bufs=4)
            for k in range(KT):
                nc.tensor.matmul(pwt[:], htT16[:, k, :],
                                 ua_sb[:, k, n2 * NB:(n2 + 1) * NB],
                                 start=(k == 0), stop=(k == KT - 1))
            nc.scalar.activation(wt_row[:, n2 * NB:(n2 + 1) * NB], pwt[:], AF.Copy)
        wtT = sb.tile([P, KT, bpc], F32, name="wtT", tag="wtT")
        for k in range(KT):
            pt2 = ps.tile([P, bpc], F32, name=f"pt2_{k}", tag="wh", bufs=4)
            nc.tensor.transpose(pt2[:], wt_row[:, k * P:(k + 1) * P], ident[:])
            nc.scalar.activation(wtT[:, k, :], pt2[:], AF.Copy)

        # ---- p_t = sigmoid(tanh(h_t @ W_p) @ v_p) * s ----
        tanhP = sb.tile([bpc, h], F32, name="tanhP", tag="tanhP")
        for n2 in range(h // NB):
            pwp = ps.tile([bpc, NB], F32, name=f"pwp_{n2}", tag="sc", bufs=4)
            for k in range(KT):
                nc.tensor.matmul(pwp[:], htT16[:, k, :],
                                 wp_sb[:, k, n2 * NB:(n2 + 1) * NB],
                                 start=(k == 0), stop=(k == KT - 1))
            nc.scalar.activation(tanhP[:, n2 * NB:(n2 + 1) * NB], pwp[:], AF.Tanh)
        z2t = sb.tile([bpc, 1], F32, name="z2t", tag="z2t")
        nc.vector.scalar_tensor_tensor(
            tanhP[:], tanhP[:], 1.0, vp_rep[:],
            op0=ALU.mult, op1=ALU.mult, accum_out=z2t[:])
        pz = ps.tile([1, bpc], F32, name="pz", tag="sc", bufs=4)
        nc.tensor.transpose(pz[:], z2t[:], ident[:])
        sg_row = sb.tile([1, bpc], F32, name="sg_row", tag="sg_row")
        nc.scalar.activation(sg_row[:], pz[:], AF.Sigmoid)
        p_row = sb.tile([1, bpc], F32, name="p_row", tag="p_row")
        nc.vector.tensor_scalar_mul(p_row[:], sg_row[:], float(s))

        # ---- watt (emitted during batch 1 prefetch; declared here) ----
        watt_holder = [None]

        def emit_watt_dma():
            watt_holder[0] = load_weight_f16("watt_sb", W_att, KT2, size,
                                             "wend", 1)

        # ---- main batch loop ----
        for b in range(bpc):
            if b + 2 < bpc:
                emit_input_dma(b + 2)
            if b == 1 or bpc <= 2:
                emit_watt_dma()
            inT = inT_tiles[b]

            sc_ps = [ps.tile([1, NB], F32, name=f"sc_{b}_{q}", tag="sc", bufs=4)
                     for q in range(SQ)]

            def emit_va_mms(hp, tanh_tiles):
                for q in range(SQ):
                    nc.tensor.matmul(sc_ps[q][:], va_sb[:, hp:hp + 1],
                                     tanh_tiles[q][:],
                                     start=(hp == 0), stop=(hp == KT - 1),
                                     skip_group_check=True)

            # v_a matmuls run one hp-group behind the main matmuls so the PE
            # never waits on ACT's tanh.
            pend = None
            for hp in range(KT):
                wh_ps = [ps.tile([P, NB], F32, name=f"wh_{b}_{hp}_{q}",
                                 tag="wh", bufs=4) for q in range(SQ)]
                for k in range(KT):
                    lhsT = wa_sb[:, k, hp * P:(hp + 1) * P]
                    for q in range(SQ):
                        nc.tensor.matmul(
                            wh_ps[q][:], lhsT, inT[:, k, q * NB:(q + 1) * NB],
                            start=(k == 0), stop=(k == KT - 1),
                            skip_group_check=True)
                if pend is not None:
                    emit_va_mms(hp - 1, pend)
                ths = []
                for q in range(SQ):
                    th = sb.tile([P, NB], F16, name=f"th_{b}_{hp}_{q}",
                                 tag="tanh", bufs=8)
                    nc.scalar.activation(th[:], wh_ps[q][:], AF.Tanh,
                                         bias=wtT[:, hp, b:b + 1])
                    ths.append(th)
                pend = ths
            emit_va_mms(KT - 1, pend)

            # ---- softmax * gaussian (rows on partition 0) ----
            score = sb.tile([1, s], F32, name=f"score_{b}", tag="score")
            for q in range(SQ):
                nc.vector.tensor_copy(score[0:1, q * NB:(q + 1) * NB],
                                      sc_ps[q][:])
            nmx = sb.tile([1, 1], F32, name=f"nmx_{b}", tag="nmx", bufs=2)
            nc.vector.tensor_reduce(nmx[:], score[:], axis=AX.X, op=ALU.max,
                                    negate=True)
            e1 = sb.tile([1, s], F16, name=f"e1_{b}", tag="e1")
            nc.scalar.activation(e1[:], score[:], AF.Exp, bias=nmx[0:1, 0:1])
            se = sb.tile([1, 1], F32, name=f"se_{b}", tag="se", bufs=2)
            nc.vector.tensor_reduce(se[:], e1[:], axis=AX.X, op=ALU.add)
            rr = sb.tile([1, 1], F32, name=f"rr_{b}", tag="rr", bufs=2)
            nc.vector.reciprocal(rr[:], se[:])
            dr = sb.tile([1, s], F16, name=f"dr_{b}", tag="gA")
            nc.vector.tensor_scalar(dr[:], pos_row[:], p_row[0:1, b:b + 1],
                                    inv_sq_denom, op0=ALU.subtract,
                                    op1=ALU.mult)
            d2 = sb.tile([1, s], F16, name=f"d2_{b}", tag="gB")
            nc.vector.tensor_mul(d2[:], dr[:], dr[:])
            gr = sb.tile([1, s], F16, name=f"gr_{b}", tag="gA")
            nc.scalar.activation(gr[:], d2[:], AF.Exp, scale=-1.0)
            wu = sb.tile([1, s], F16, name=f"wu_{b}", tag="gB")
            nc.vector.scalar_tensor_tensor(wu[:], e1[:], rr[0:1, 0:1], gr[:],
                                           op0=ALU.mult, op1=ALU.mult)

            # ---- replicate weights row across partitions ----
            wrep = sb.tile([P, s], F16, name=f"wrep_{b}", tag="wrep", bufs=2)
            for q in range(SQ):
                pwr = ps.tile([P, NB], F32, name=f"pwr_{b}_{q}", tag="sc",
                              bufs=4)
                nc.tensor.matmul(pwr[:], ones1[0:1, :],
                                 wu[0:1, q * NB:(q + 1) * NB],
                                 start=True, stop=True, skip_group_check=True)
                nc.scalar.activation(wrep[:, q * NB:(q + 1) * NB], pwr[:],
                                     AF.Copy)

            # ---- context^T via fused multiply+reduce on DVE ----
            ctxa = sb.tile([P, KT], F32, name=f"ctxa_{b}", tag="ctxa", bufs=2)
            for k in range(KT):
                nc.vector.scalar_tensor_tensor(
                    inT[:, k, :], inT[:, k, :], 1.0, wrep[:],
                    op0=ALU.mult, op1=ALU.mult,
                    accum_out=ctxa[:, k:k + 1])
                nc.vector.tensor_copy(combT[:, k, b:b + 1], ctxa[:, k:k + 1])

        # ---- final projection: tanh([ctx, h_t] @ W_att) ----
        watt_sb = watt_holder[0]
        outsb = sb.tile([bpc, size], F32, name="outsb", tag="outsb")
        pfs = [ps.tile([bpc, NB], F32, name=f"pf_{n2}", tag="sc", bufs=4)
               for n2 in range(NO)]
        for kk in list(range(KT, KT2)) + list(range(KT)):
            for n2 in range(NO):
                nc.tensor.matmul(pfs[n2][:], combT[:, kk, :],
                                 watt_sb[:, kk, n2 * NB:(n2 + 1) * NB],
                                 start=(kk == KT), stop=(kk == KT - 1),
                                 skip_group_check=True)
        for n2 in range(NO):
            nc.scalar.activation(outsb[:, n2 * NB:(n2 + 1) * NB], pfs[n2][:],
                                 AF.Tanh)
        nc.scalar.dma_start(out[:], outsb[:])

    nc.compile()
    return nc


def kernel(**inputs):
    global _compiled
    from concourse import bass_utils

    if _compiled is None:
        _compiled = _build()

    x = np.ascontiguousarray(np.asarray(inputs["inputs"], dtype=np.float32))
    weights = {
        k: np.ascontiguousarray(np.asarray(inputs[k], dtype=np.float32))
        for k in ("W_p", "v_p", "W_a", "U_a", "v_a", "W_att")
    }
    in_maps = [
        {"inputs": x[i * BPC:(i + 1) * BPC], **weights} for i in range(N_CORES)
    ]
    res = bass_utils.run_bass_kernel_spmd(_compiled, in_maps,
                                          list(range(N_CORES)))
    return np.concatenate([res.results[i]["out"] for i in range(N_CORES)],
                          axis=0).astype(np.float32)



# revision 13
# speedup vs baseline: 1.0364x; 1.0364x over previous
"""Trainium2 Bass kernel: Bahdanau local-p attention (B=32, S=2048, H=1024).

Sharding: data-parallel over batch. Each of the 8 cores processes B/8 = 4
batches end-to-end (weights replicated); no collectives.

Per-core dataflow (fp8e4 DoubleRow matmuls, fp32 PSUM):
  1. x[b] fp32 -> fp8e4 cast DMA (SWDGE) into DRAM; u16-viewed xbar
     transposes -> SBUF xt [128, 4j, 2048] u16, pair-interleaved:
     partition p, block j holds bytes (h=256j+2p, h=256j+2p+1) along s.
  2. main matmul in DoubleRow fp8: lhsT = W_a tile [128,2,128]
     (h = 256j+2p+i layout), rhs = interleaved fp8 view [128,2(s1B),N(s2B)].
  3. tanh(WH^T + U_a h_t) on ACT (per-partition bias), fp8 out.
  4. score via DoubleRow v_a matmuls; the q-th s-block accumulates into
     row q of one [16,512] PSUM bank (v_a placed in weight column q).
  5. softmax (no max-sub; scores are O(1)) * gaussian on [4,512] rows;
     weights row scaled by 256 for fp8 range.
  6. weights replicated to 128 partitions via ones-matmul; context via
     8 strided fp8 DVE multiply-accumulates against xt (one per (j,i)).
  7. final tanh([ctx, h_t] @ W_att) in fp16; W_att ctx-rows loaded in the
     matching (j p two) permuted order.
"""

import math
from contextlib import ExitStack

import numpy as np

B, S, H, SIZE = 32, 2048, 1024, 1024
N_CORES = 8
BPC = B // N_CORES
P = 128
NB = 512

_compiled = None


def _build(bpc=BPC, s=S, h=H, size=SIZE, debug=False):
    import concourse.bacc as bacc
    import concourse.mybir as mybir
    import concourse.tile as tile

    F32 = mybir.dt.float32
    F16 = mybir.dt.float16
    F8 = mybir.dt.float8e4
    U16 = mybir.dt.uint16
    AF = mybir.ActivationFunctionType
    ALU = mybir.AluOpType
    AX = mybir.AxisListType
    DR = mybir.MatmulPerfMode.DoubleRow

    KT = h // P            # 8 h-tiles of 128
    NJ = h // 256          # 4 k-blocks of 256 (DoubleRow groups)
    SQ = s // NB           # 4 s-blocks of 512
    KT2 = 2 * h // P       # 16 k-tiles for the final projection
    NO = size // NB        # 2 output blocks of 512
    denom = 2.0 * ((s // 2) / 2.0) ** 2
    inv_sq_denom = 1.0 / math.sqrt(denom)
    WSC = 256.0            # fp8 scale for the attention-weights row

    nc = bacc.Bacc("TRN2", target_bir_lowering=False, debug=debug)

    x = nc.dram_tensor("inputs", [bpc, s, h], F32, kind="ExternalInput").ap()
    W_p = nc.dram_tensor("W_p", [h, h], F32, kind="ExternalInput").ap()
    v_p = nc.dram_tensor("v_p", [h, 1], F32, kind="ExternalInput").ap()
    W_a = nc.dram_tensor("W_a", [h, h], F32, kind="ExternalInput").ap()
    U_a = nc.dram_tensor("U_a", [h, h], F32, kind="ExternalInput").ap()
    v_a = nc.dram_tensor("v_a", [h, 1], F32, kind="ExternalInput").ap()
    W_att = nc.dram_tensor("W_att", [2 * h, size], F32, kind="ExternalInput").ap()
    out = nc.dram_tensor("out", [bpc, size], F32, kind="ExternalOutput").ap()

    with tile.TileContext(nc) as tc, ExitStack() as ctx:
        dp = ctx.enter_context(tc.tile_pool(name="dram", bufs=3, space="DRAM"))
        sb = ctx.enter_context(tc.tile_pool(name="sb", bufs=1))
        ps = ctx.enter_context(tc.tile_pool(name="ps", bufs=1, space="PSUM"))
        xf8 = [None] * bpc

        # ---- SWDGE queue: wa8, xf8[0] (split), xf8[1], xf8[2], xf8[3],
        # watt16 comes from the batch loop. HWDGE sync queue: xbar
        # transposes ONLY. HWDGE scalar queue: plain fp32 loads.
        wa8 = sb.tile([P, NJ, 2, h], F8, name="wa8", tag="wa8")
        nc.gpsimd.dma_start(
            wa8[:], W_a.rearrange("(j p two) m -> p j two m", j=NJ, p=P))

        xt_tiles = [None] * bpc

        def emit_cast(b, nsplit=1):
            xf8[b] = dp.tile([s, h], F8, name=f"xf8_{b}", tag="xf8")
            for c in range(nsplit):
                sc_ = s // nsplit
                nc.gpsimd.dma_start(
                    xf8[b][c * sc_:(c + 1) * sc_, :],
                    x[b, c * sc_:(c + 1) * sc_, :])

        def emit_transposes(b, nsplit=1):
            xt = sb.tile([P, NJ, s], U16, name=f"xt_{b}", tag="xt", bufs=2)
            xu = xf8[b][:].bitcast(U16)  # [s, h//2]
            for j in range(NJ):
                for c in range(nsplit):
                    sc_ = s // nsplit
                    nc.sync.dma_start(
                        xt[:, j, c * sc_:(c + 1) * sc_],
                        xu[c * sc_:(c + 1) * sc_, j * P:(j + 1) * P],
                        transpose=True)
            xt_tiles[b] = xt

        emit_cast(0, nsplit=2)
        emit_cast(1)

        # ---- plain fp32 loads on the scalar HWDGE queue ----
        htb = sb.tile([bpc, h], F32, name="htb", tag="htb")
        nc.scalar.dma_start(htb[:], x[:, s - 1, :])
        uaw_halves = []
        for wname, wap in (("ua", U_a), ("wp", W_p)):
            for hh in range(2):
                st = sb.tile([P, KT // 2, h], F32, name=f"{wname}32_{hh}",
                             tag="uw32", bufs=2)
                nc.scalar.dma_start(
                    st[:], wap.rearrange("(k p) n -> p k n", p=P)
                    [:, hh * (KT // 2):(hh + 1) * (KT // 2), :])
                uaw_halves.append(st)
        vp_rep = sb.tile([bpc, h], F32, name="vp_rep", tag="vp_rep")
        for i in range(bpc):
            nc.scalar.dma_start(vp_rep[i:i + 1, :], v_p.rearrange("n o -> o n"))
        va32 = sb.tile([P, KT, 1], F32, name="va32", tag="va32")
        nc.scalar.dma_start(va32[:], v_a.rearrange("(a p) o -> p a o", p=P))

        # ---- on-chip weight casts (DVE) ----
        ua16 = sb.tile([P, KT, h], F16, name="ua16", tag="uw16", bufs=2)
        wp16 = sb.tile([P, KT, h], F16, name="wp16", tag="uw16", bufs=2)
        for wi, w16 in ((0, ua16), (1, wp16)):
            for hh in range(2):
                nc.vector.tensor_copy(
                    w16[:, hh * (KT // 2):(hh + 1) * (KT // 2), :],
                    uaw_halves[2 * wi + hh][:])
        # v_a into weight-column q of a [P, KT, 16] tile per q (score rows)
        vaq = sb.tile([P, KT, 16], F8, name="vaq", tag="vaq", bufs=4)
        vaqs = [vaq]
        for q in range(1, SQ):
            vaqs.append(sb.tile([P, KT, 16], F8, name=f"vaq{q}", tag="vaq",
                                bufs=4))
        for q in range(SQ):
            nc.vector.memset(vaqs[q][:], 0.0)
            nc.vector.tensor_copy(vaqs[q][:, :, q:q + 1], va32[:])

        # ---- constants ----
        ident = sb.tile([bpc, bpc], F32, name="ident", tag="ident")
        nc.gpsimd.iota(ident[:], pattern=[[1, bpc]], base=0,
                       channel_multiplier=-1,
                       allow_small_or_imprecise_dtypes=True)
        nc.vector.tensor_scalar(ident[:], ident[:], 0.0, None,
                                op0=ALU.is_equal)
        ones1 = sb.tile([bpc, P], F16, name="ones1", tag="ones1")
        nc.vector.memset(ones1[:], 1.0)
        chan4 = sb.tile([SQ, P], F32, name="chan4", tag="chan4")
        nc.gpsimd.iota(chan4[:], pattern=[[0, P]], base=0,
                       channel_multiplier=1,
                       allow_small_or_imprecise_dtypes=True)
        sels = []
        for q in range(SQ):
            sq = sb.tile([SQ, P], F16, name=f"sel{q}", tag="sel", bufs=SQ)
            nc.vector.tensor_scalar(sq[:], chan4[:], float(q), None,
                                    op0=ALU.is_equal)
            sels.append(sq)
        onesc = sb.tile([1, bpc], F16, name="onesc", tag="onesc")
        nc.vector.memset(onesc[:], 1.0)
        pos4 = sb.tile([SQ, NB], F16, name="pos4", tag="pos")
        nc.gpsimd.iota(pos4[:], pattern=[[1, NB]], base=0,
                       channel_multiplier=NB,
                       allow_small_or_imprecise_dtypes=True)

        # ---- h_t transposes -> htT16 [P, KT, bpc], combT h_t half ----
        htT16 = sb.tile([P, KT, bpc], F16, name="htT16", tag="htT16")
        combT = sb.tile([P, KT2, bpc], F16, name="combT", tag="combT")
        for k in range(KT):
            pt = ps.tile([P, bpc], F32, name=f"pt_{k}", tag="misc", bufs=1)
            nc.tensor.transpose(pt[:], htb[:, k * P:(k + 1) * P], ident[:])
            nc.vector.tensor_copy(htT16[:, k, :], pt[:])
            nc.vector.tensor_copy(combT[:, KT + k, :], pt[:])

        # ---- wt = h_t @ U_a -> wtT [P, KT, bpc] fp32 (tanh bias) ----
        wt_row = sb.tile([bpc, h], F32, name="wt_row", tag="wt_row")
        for n2 in range(h // NB):
            pwt = ps.tile([bpc, NB], F32, name=f"pwt_{n2}", tag="pk", bufs=2)
            for k in range(KT):
                nc.tensor.matmul(pwt[:], htT16[:, k, :],
                                 ua16[:, k, n2 * NB:(n2 + 1) * NB],
                                 start=(k == 0), stop=(k == KT - 1))
            nc.scalar.activation(wt_row[:, n2 * NB:(n2 + 1) * NB], pwt[:],
                                 AF.Copy)
        wtT = sb.tile([P, KT, bpc], F32, name="wtT", tag="wtT")
        for k in range(KT):
            pt2 = ps.tile([P, bpc], F32, name=f"pt2_{k}", tag="misc", bufs=1)
            nc.tensor.transpose(pt2[:], wt_row[:, k * P:(k + 1) * P], ident[:])
            nc.scalar.activation(wtT[:, k, :], pt2[:], AF.Copy)

        # ---- p_t = sigmoid(tanh(h_t @ W_p) @ v_p) * s -> pbT [bpc, bpc] ----
        tanhP = sb.tile([bpc, h], F32, name="tanhP", tag="tanhP")
        for n2 in range(h // NB):
            pwp = ps.tile([bpc, NB], F32, name=f"pwp_{n2}", tag="pk", bufs=2)
            for k in range(KT):
                nc.tensor.matmul(pwp[:], htT16[:, k, :],
                                 wp16[:, k, n2 * NB:(n2 + 1) * NB],
                                 start=(k == 0), stop=(k == KT - 1))
            nc.scalar.activation(tanhP[:, n2 * NB:(n2 + 1) * NB], pwp[:],
                                 AF.Tanh)
        z2t = sb.tile([bpc, 1], F32, name="z2t", tag="z2t")
        nc.vector.scalar_tensor_tensor(
            tanhP[:], tanhP[:], 1.0, vp_rep[:],
            op0=ALU.mult, op1=ALU.mult, accum_out=z2t[:])
        pz = ps.tile([1, bpc], F32, name="pz", tag="misc", bufs=1)
        nc.tensor.transpose(pz[:], z2t[:], ident[:])
        p_row = sb.tile([1, bpc], F16, name="p_row", tag="p_row")
        nc.scalar.activation(p_row[:], pz[:], AF.Sigmoid)
        pbt_ps = ps.tile([bpc, bpc], F32, name="pbt_ps", tag="misc", bufs=1)
        nc.tensor.matmul(pbt_ps[:], onesc[:], p_row[:], start=True, stop=True)
        pbT = sb.tile([bpc, bpc], F32, name="pbT", tag="pbT")
        nc.scalar.activation(pbT[:], pbt_ps[:], AF.Copy, scale=float(s))

        emit_transposes(0, nsplit=2)
        emit_transposes(1)

        # ---- W_att fp16 (emitted during the batch loop) ----
        watt_holder = [None]

        def emit_watt():
            # ctx rows (0..h) in the pair-permuted order, h_t rows standard
            w16 = sb.tile([P, KT2, size], F16, name="watt16", tag="watt")
            nc.gpsimd.dma_start(
                w16[:, 0:KT, :],
                W_att[0:h, :].rearrange("(j p two) n -> p j two n",
                                        j=NJ, p=P))
            nc.gpsimd.dma_start(
                w16[:, KT:KT2, :],
                W_att[h:2 * h, :].rearrange("(k p) n -> p k n", p=P))
            watt_holder[0] = w16

        # ---- main batch loop ----
        for b in range(bpc):
            if b + 2 < bpc:
                emit_cast(b + 2)
                emit_transposes(b + 2)
            if b == 1:
                emit_watt()
            xt = xt_tiles[b]
            # interleaved fp8 rhs views per j: [P, 2(s1B), s(s2B)]
            rhs_j = [(xt[:, j, :].bitcast(F8)
                      .rearrange("p (n two) -> p n two", two=2)
                      .rearrange("p n two -> p two n"))
                     for j in range(NJ)]

            th8 = sb.tile([P, KT, s], F8, name=f"th_{b}", tag="tanh", bufs=1)
            sc_ps = ps.tile([16, NB], F32, name=f"sc_{b}", tag="sc", bufs=1)

            def emit_va_mms(a):
                for q in range(SQ):
                    nc.tensor.matmul(
                        sc_ps[:], vaqs[q][:, 2 * a:2 * a + 2, :],
                        th8[:, 2 * a:2 * a + 2, q * NB:(q + 1) * NB],
                        start=(a == 0 and q == 0),
                        stop=(a == KT // 2 - 1 and q == SQ - 1),
                        perf_mode=DR, skip_group_check=True)

            for hp in range(KT):
                for sh in range(2):
                    wh = ps.tile([P, 2 * NB], F32, name=f"wh_{b}_{hp}_{sh}",
                                 tag="wh", bufs=2)
                    for j in range(NJ):
                        lhsT = wa8[:, j, :, hp * P:(hp + 1) * P]
                        for q2 in range(2):
                            s0 = sh * 2 * NB + q2 * NB
                            nc.tensor.matmul(
                                wh[:, q2 * NB:(q2 + 1) * NB], lhsT,
                                rhs_j[j][:, :, s0:s0 + NB],
                                start=(j == 0), stop=(j == NJ - 1),
                                perf_mode=DR, skip_group_check=True)
                    nc.scalar.activation(
                        th8[:, hp, sh * 2 * NB:(sh + 1) * 2 * NB], wh[:],
                        AF.Tanh, bias=wtT[:, hp, b:b + 1])
                if hp % 2 == 1 and hp >= 3:
                    emit_va_mms(hp // 2 - 1)
            emit_va_mms(KT // 2 - 1)

            # ---- softmax * gaussian on [4, 512] rows ----
            score4 = sc_ps[0:SQ, :]
            e4 = sb.tile([SQ, NB], F16, name=f"e4_{b}", tag="e4", bufs=2)
            nc.scalar.activation(e4[:], score4, AF.Exp)
            zp = sb.tile([SQ, 1], F16, name=f"zp_{b}", tag="zp", bufs=2)
            with nc.allow_low_precision(reason="Z fits fp16 comfortably"):
                nc.vector.tensor_reduce(zp[:], e4[:], axis=AX.X, op=ALU.add)
            zs_ps = ps.tile([1, 1], F32, name=f"zs_{b}", tag="misc", bufs=1)
            nc.tensor.matmul(zs_ps[:], zp[:], ones1[0:SQ, 0:1],
                             start=True, stop=True, skip_group_check=True)
            rr = sb.tile([1, 1], F16, name=f"rr_{b}", tag="rr", bufs=2)
            with nc.allow_low_precision(reason="1/Z fits fp16"):
                nc.vector.reciprocal(rr[:], zs_ps[:])
            rr_ps = ps.tile([SQ, 1], F32, name=f"rrp_{b}", tag="misc", bufs=1)
            nc.tensor.matmul(rr_ps[:], onesc[0:1, 0:SQ], rr[:],
                             start=True, stop=True, skip_group_check=True)
            rr4 = sb.tile([SQ, 1], F32, name=f"rr4_{b}", tag="rr4", bufs=2)
            nc.scalar.activation(rr4[:], rr_ps[:], AF.Copy, scale=WSC)
            t4 = sb.tile([SQ, NB], F16, name=f"t4_{b}", tag="t4", bufs=2)
            nc.vector.tensor_scalar(t4[:], pos4[:], pbT[0:SQ, b:b + 1],
                                    inv_sq_denom, op0=ALU.subtract,
                                    op1=ALU.mult)
            d2n = sb.tile([SQ, NB], F16, name=f"d2_{b}", tag="d2", bufs=2)
            nc.vector.scalar_tensor_tensor(d2n[:], t4[:], -1.0, t4[:],
                                           op0=ALU.mult, op1=ALU.mult)
            gr = sb.tile([SQ, NB], F16, name=f"gr_{b}", tag="gr", bufs=2)
            nc.scalar.activation(gr[:], d2n[:], AF.Exp)
            wu4 = sb.tile([SQ, NB], F8, name=f"wu_{b}", tag="wu", bufs=2)
            nc.vector.scalar_tensor_tensor(wu4[:], e4[:], rr4[:], gr[:],
                                           op0=ALU.mult, op1=ALU.mult)

            # ---- replicate weight rows to 128 partitions (x WSC) ----
            wrep = sb.tile([P, s], F8, name=f"wrep_{b}", tag="wrep", bufs=2)
            for q in range(SQ):
                pwr = ps.tile([P, NB], F32, name=f"pwr_{b}_{q}", tag="pk",
                              bufs=2)
                nc.tensor.matmul(pwr[:], sels[q][:], wu4[:],
                                 start=True, stop=True, skip_group_check=True)
                nc.scalar.activation(wrep[:, q * NB:(q + 1) * NB], pwr[:],
                                     AF.Copy)

            # ---- context: 8 strided fp8 multiply-accumulates ----
            junk = sb.tile([P, s], F8, name=f"junk_{b}", tag="junk", bufs=2)
            ctxa = sb.tile([P, NJ, 2], F32, name=f"ctxa_{b}", tag="ctxa",
                           bufs=2)
            for j in range(NJ):
                xv = xt[:, j, :].bitcast(F8).rearrange(
                    "p (n two) -> p n two", two=2)
                for i in range(2):
                    nc.vector.scalar_tensor_tensor(
                        junk[:], xv[:, :, i], 1.0, wrep[:],
                        op0=ALU.mult, op1=ALU.mult,
                        accum_out=ctxa[:, j, i:i + 1])
            nc.vector.tensor_scalar_mul(
                combT[:, 0:KT, b:b + 1].rearrange("p k o -> p (k o)"),
                ctxa[:].rearrange("p j i -> p (j i)"), 1.0 / WSC)

        # ---- final projection: tanh([ctx, h_t] @ W_att) ----
        watt16 = watt_holder[0]
        outsb = sb.tile([bpc, size], F32, name="outsb", tag="outsb")
        pfs = [ps.tile([bpc, NB], F32, name=f"pf_{n2}", tag="pk", bufs=2)
               for n2 in range(NO)]
        for kk in range(KT2):
            for n2 in range(NO):
                nc.tensor.matmul(pfs[n2][:], combT[:, kk, :],
                                 watt16[:, kk, n2 * NB:(n2 + 1) * NB],
                                 start=(kk == 0), stop=(kk == KT2 - 1),
                                 skip_group_check=True)
        for n2 in range(NO):
            nc.scalar.activation(outsb[:, n2 * NB:(n2 + 1) * NB], pfs[n2][:],
                                 AF.Tanh)
        nc.scalar.dma_start(out[:], outsb[:])

    nc.compile()
    return nc


def kernel(**inputs):
    global _compiled
    from concourse import bass_utils

    if _compiled is None:
        _compiled = _build()

    x = np.ascontiguousarray(np.asarray(inputs["inputs"], dtype=np.float32))
    weights = {
        k: np.ascontiguousarray(np.asarray(inputs[k], dtype=np.float32))
        for k in ("W_p", "v_p", "W_a", "U_a", "v_a", "W_att")
    }
    in_maps = [
        {"inputs": x[i * BPC:(i + 1) * BPC], **weights} for i in range(N_CORES)
    ]
    res = bass_utils.run_bass_kernel_spmd(_compiled, in_maps,
                                          list(range(N_CORES)))
    return np.concatenate([res.results[i]["out"] for i in range(N_CORES)],
                          axis=0).astype(np.float32)
